# revision 23
# baseline (speedup 1.0000x reference)
"""Multi-head attention (softmax over the HEADS axis) on 8 trn2 NeuronCores.

Reference math (B=2, S=2048, D=512, H=8, Dk=64):
    q = split_heads(Q @ w_q.T + b_q)          # [B,H,S,Dk]
    scores = q @ k.T / sqrt(Dk)               # [B,H,Sq,Sk]
    probs = softmax(scores, axis=1)           # softmax over H (source quirk!)
    attn = probs @ v                          # [B,H,Sq,Dk]
    out = concat_heads(attn) @ w_o.T + b_o    # [B,S,D]

Because softmax is over H, it is local to each (b, sq, sk) position: sharding
over (batch x query rows) needs no cross-core communication.  Core c handles
batch c//4, query rows (c%4)*512 .. +512, with all 8 heads resident.

v2 schedule: all four engines carry ~70-90us of true work, so the kernel is
organized so each engine's in-order queue never blocks another's:
  - K projection is kc-major; only Q + K's first kj block run before the
    t loop, so exp starts ~7us in (vs ~17us when V sat ahead of scores).
  - Remaining K blocks and all V projection pairs are interleaved INTO the
    t loop (one [128,2,512] psum tile each), their PSUM->SBUF copies go to
    gpsimd/scalar -- never vector, whose queue carries the softmax chain.
  - Per tile, attn matmuls of tile t-2 are emitted BETWEEN score matmul
    groups so the PE always has work while ACT drains scores banks.
  - ACT does exp only (plus a few interleaved copies); head-sum tree is
    split gpsimd (L1a) / vector (L1b, L2, L3); fast-reciprocal and a single
    broadcast normalize-mul ([128,8,512] * r with 0-stride head dim) on DVE
    keep the vector queue at ~4.6us/tile in 2x (16-bit packed) mode.
  - Drain: attn tail, per-chunk attnT copies on separate engines, O-proj
    accumulated c-outer so it starts as soon as the first chunk lands.

Layouts (contraction dim always on SBUF partitions):
  qt/kt/vt  [128, 4, S*]  = X.T      (din = chunk*128 + p), bf16
  w*t       [128, 4, 512] = W.T      (din = chunk*128 + p), bf16
  qTs/kTs   [128, 4, S*]  = proj.T   (dout = m*128 + p), bf16
  vs        [128, 16, 512] = v natural (kj on partitions), bf16
  scores    psum [kj=128, 2, 512qi] per head pair -> exp -> softmax over h
  attn      psum [2*64=128 d, 512 qi] per head-pair, accumulated over kj tiles
  out       [qi, 512] natural, fp32
"""

import numpy as np

B, S, D, H, DK = 2, 2048, 512, 8, 64
NCORES = 8
CPB = NCORES // B          # cores per batch
QI = S // CPB              # query rows per core (512)
KJT = 128                  # kj tile (partition dim of scores)
NKJ = S // KJT             # 16 kj tiles
NC_, CH = 128, 4           # partitions, din chunks
SCALE = 1.0 / np.sqrt(DK)  # folded into exp activation
LAG = 2                    # attn matmuls run LAG tiles behind the softmax


def _chunk(x, dt):
    """[512, F] -> [128, 4, F] with row = chunk*128 + p."""
    f = x.shape[1]
    return np.ascontiguousarray(
        np.ascontiguousarray(x).reshape(CH, NC_, f).transpose(1, 0, 2)
    ).astype(dt)


def _recip_sum_op():
    """Register (once) a fused custom-DVE op: out = 1/(Src0+Src1), one
    Newton step off the BITWISE_NOT seed (6 ALU stages, ~0.4% max err --
    bf16-comparable).  Saves the separate L3 add each tile.  Returns the
    DveOp, or None if registration fails (caller falls back to add+recip)."""
    try:
        import numpy as np

        from concourse import dve_ops as dops
        from concourse.dve_spec import AluOp, Bin, Spec, lower
        from concourse.dve_uop import DveOpSpec
        from concourse.dve_spec import C0, C1, Src0, Src1

        name = "RECIP_SUM_ANT"
        for op in dops.OPS:
            if op.name == name:
                return op

        _X = Src0 + Src1
        _nx = Bin(AluOp.BITWISE_NOT, _X, _X)
        _y0 = _nx * C0

        def _ref(in0, in1, c0, c1, c2):
            x = (in0.astype(np.float32) + in1.astype(np.float32)).astype(
                np.float32
            )
            nx = (~x.view(np.int32)).view(np.float32)
            y0 = nx * c0
            return y0 * (c1 - x * y0)

        spec = Spec(body=_y0 * (C1 - _X * _y0), reference=_ref)
        row = max(dops._SUB_OPCODE_FOR_NAME.values()) + 1
        assert row < 0x20
        shas = {}
        for ver in ("v3", "v4"):
            tmp = DveOpSpec(
                name=name, opcode=row, uops=lower(spec, ver=ver), rd1_en=True
            )
            shas[ver] = tmp.sha(ver)
        op = dops.DveOp(name, spec, subdim=False, uops_sha=shas)
        dops.OPS.append(op)
        dops.CUSTOM_DVE_SPECS[name] = spec
        dops._SUB_OPCODE_FOR_NAME[name] = row
        return op
    except Exception:
        return None


def _build(with_bias):
    from contextlib import ExitStack

    import concourse.bass as bass
    import concourse.mybir as mybir
    import concourse.tile as tile
    from concourse import bacc

    fp32 = mybir.dt.float32
    bf16 = mybir.dt.bfloat16

    nc = bacc.Bacc(
        "TRN2",
        target_bir_lowering=False,
        debug=False,
        enable_asserts=False,
        num_devices=NCORES,
    )

    def din(name, shape):
        return nc.dram_tensor(name, shape, bf16, kind="ExternalInput").ap()

    qt_d = din("qt", [NC_, CH, QI])
    kt_d = din("kt", [NC_, CH, S])
    vt_d = din("vt", [NC_, CH, S])
    w_d = {n: din(n, [NC_, CH, D]) for n in ("wqt", "wkt", "wvt", "wot")}
    if with_bias:
        b_d = {n: din(n, [1, D]) for n in ("bq", "bk", "bv", "bo")}
    out_d = nc.dram_tensor("out", [QI, D], fp32, kind="ExternalOutput").ap()

    with tile.TileContext(nc) as tc, ExitStack() as ctx:
        acts = ctx.enter_context(tc.tile_pool(name="acts", bufs=1))
        sm = ctx.enter_context(tc.tile_pool(name="sm", bufs=4))
        pp = ctx.enter_context(tc.tile_pool(name="pp", bufs=4))
        ps = ctx.enter_context(tc.tile_pool(name="ps", bufs=2, space="PSUM"))
        pa = ctx.enter_context(tc.tile_pool(name="pa", bufs=4, space="PSUM"))

        qTs = acts.tile([NC_, CH, QI], bf16, tag="qTs")
        kTs = acts.tile([NC_, CH, S], bf16, tag="kTs")
        vs = acts.tile([NC_, NKJ, D], bf16, tag="vs")
        attnT = acts.tile([NC_, CH, QI], bf16, tag="attnT")
        outsb = acts.tile([NC_, CH, D], fp32, tag="outsb")
        qraw = acts.tile([NC_, CH, QI], bf16, tag="qraw")
        kraw = acts.tile([NC_, CH, S], bf16, tag="kraw")
        vraw = acts.tile([NC_, CH, S], bf16, tag="vraw")
        wsb = {}
        for n in ("wqt", "wkt", "wvt", "wot"):
            wsb[n] = acts.tile([NC_, CH, D], bf16, tag=n, name=n)

        if with_bias:
            ones = acts.tile([1, D], bf16, tag="ones")
            nc.vector.memset(ones, 1.0)
            brow = {}
            for n in ("bq", "bk", "bv", "bo"):
                brow[n] = acts.tile([1, D], bf16, tag=n, name=n)
                nc.sync.dma_start(out=brow[n], in_=b_d[n])

        # ---- input DMAs, ordered by first use ----
        nc.sync.dma_start(out=wsb["wqt"], in_=w_d["wqt"])
        nc.sync.dma_start(out=qraw, in_=qt_d)
        nc.sync.dma_start(out=wsb["wkt"], in_=w_d["wkt"])

        def kv_chunk_dma(raw, dram, blk):
            sl = slice(blk * 512, (blk + 1) * 512)
            for c in range(CH):
                nc.sync.dma_start(out=raw[:, c, sl], in_=dram[:, c, sl])

        kv_chunk_dma(kraw, kt_d, 0)
        nc.sync.dma_start(out=wsb["wvt"], in_=w_d["wvt"])
        kv_chunk_dma(vraw, vt_d, 0)
        for blk in range(1, 4):
            kv_chunk_dma(kraw, kt_d, blk)
            kv_chunk_dma(vraw, vt_d, blk)
        nc.sync.dma_start(out=wsb["wot"], in_=w_d["wot"])

        def copy_to(eng, dst, src):
            if eng is nc.scalar:
                eng.copy(dst, src)
            else:
                eng.tensor_copy(dst, src)

        def bias_mm(pt_ap, bname, col_slice):
            """rank-1 bias init: psum = bias-row (x) ones-row (or flipped)."""
            if col_slice is not None:  # bias along partitions
                lhsT = brow[bname][:, col_slice]
                rhs = ones[:, : pt_ap.shape[-1]]
            else:  # bias along free dim
                lhsT = ones[:, :128]
                rhs = brow[bname]
            nc.tensor.matmul(pt_ap, lhsT=lhsT, rhs=rhs, start=True, stop=False)

        def qproj_mp(mp):
            pt = ps.tile([NC_, 2, 512], fp32, tag="ps", name=f"qp{mp}")
            for j in (0, 1):
                m = 2 * mp + j
                if with_bias:
                    bias_mm(pt[:, j, :QI], "bq", slice(m * 128, (m + 1) * 128))
                for c in range(CH):
                    nc.tensor.matmul(
                        pt[:, j, :QI],
                        lhsT=wsb["wqt"][:, c, m * 128 : (m + 1) * 128],
                        rhs=qraw[:, c, :],
                        start=(c == 0 and not with_bias),
                        stop=(c == CH - 1),
                    )
            return qTs[:, 2 * mp : 2 * mp + 2, :], pt[:, :, :QI]

        def kproj_mp(kc, mp):
            sl = slice(kc * 512, (kc + 1) * 512)
            pt = ps.tile([NC_, 2, 512], fp32, tag="ps", name=f"kp{kc}_{mp}")
            for j in (0, 1):
                m = 2 * mp + j
                if with_bias:
                    bias_mm(pt[:, j, :], "bk", slice(m * 128, (m + 1) * 128))
                for c in range(CH):
                    nc.tensor.matmul(
                        pt[:, j, :],
                        lhsT=wsb["wkt"][:, c, m * 128 : (m + 1) * 128],
                        rhs=kraw[:, c, sl],
                        start=(c == 0 and not with_bias),
                        stop=(c == CH - 1),
                    )
            return kTs[:, 2 * mp : 2 * mp + 2, sl], pt

        def vproj_pair(p):
            pt = ps.tile([NC_, 2, 512], fp32, tag="ps", name=f"vp{p}")
            for j in (0, 1):
                td = 2 * p + j
                if with_bias:
                    bias_mm(pt[:, j, :], "bv", None)
                for c in range(CH):
                    nc.tensor.matmul(
                        pt[:, j, :],
                        lhsT=vraw[:, c, td * 128 : (td + 1) * 128],
                        rhs=wsb["wvt"][:, c, :],
                        start=(c == 0 and not with_bias),
                        stop=(c == CH - 1),
                    )
            return vs[:, 2 * p : 2 * p + 2, :], pt

        # ---- prologue: Q, K kc-block 0 (vector is idle here; use it) ----
        # (gpsimd can NOT read PSUM -- all psum->sbuf copies on scalar/vector)
        for mp, eng in ((0, nc.vector), (1, nc.scalar)):
            dst, src = qproj_mp(mp)
            copy_to(eng, dst, src)
        for mp, eng in ((0, nc.vector), (1, nc.scalar)):
            dst, src = kproj_mp(0, mp)
            copy_to(eng, dst, src)

        from concourse.dve_ops import (
            RECIP_APPROX_FAST_CONSTS as _RC,
            RECIPROCAL_APPROX_FAST as _RF,
        )

        RSUM = _recip_sum_op()

        # attn psum: tile dc holds heads 2dc (p 0..63), 2dc+1 (p 64..127)
        at = [pa.tile([NC_, 512], fp32, tag="attn", name=f"at{i}") for i in range(4)]
        prs_hist = {}

        def emit_attn_half(td, half):
            pr = prs_hist[td]
            for h in range(4 * half, 4 * half + 4):
                po = (h % 2) * 64
                nc.tensor.matmul(
                    at[h // 2][po : po + 64, :QI],
                    lhsT=vs[:, td, h * 64 : (h + 1) * 64],
                    rhs=pr[:, h, :],
                    start=(td == 0),
                    stop=(td == NKJ - 1),
                )

        # background projection work interleaved into the t loop:
        # ("K", kc, mp, eng) or ("V", p, eng); copies on gpsimd/scalar only.
        # one background-projection tile per loop iteration; K block kc feeds
        # scores t=4kc.., V pair p feeds attn(2p) emitted at t=2p+2.  All
        # copies on scalar, emitted between exp m1 and exp m2 so the copy
        # lands before the PE reaches scores m3 (whose psum slot it gates).
        BG = {
            0: ("K", 1, 0),
            1: ("V", 0),
            2: ("K", 1, 1),
            3: ("V", 1),
            4: ("K", 2, 0),
            5: ("V", 2),
            6: ("K", 2, 1),
            7: ("V", 3),
            8: ("K", 3, 0),
            9: ("V", 4),
            10: ("K", 3, 1),
            11: ("V", 5),
            12: ("V", 6),
            13: ("V", 7),
        }

        def emit_bg_mms(job):
            if job[0] == "K":
                return kproj_mp(job[1], job[2])
            return vproj_pair(job[1])

        for t in range(NKJ):
            bg = BG.get(t)
            exp_t = sm.tile([NC_, H, QI], bf16, tag="exp", bufs=5, name=f"ex{t}")

            def scores_m(m):
                spt = ps.tile([NC_, 2, 512], fp32, tag="ps", name=f"s{t}_{m}")
                for j in (0, 1):
                    po = j * 64
                    nc.tensor.matmul(
                        spt[:, j, :QI],
                        lhsT=kTs[po : po + 64, m, t * 128 : (t + 1) * 128],
                        rhs=qTs[po : po + 64, m, :],
                        start=True,
                        stop=True,
                    )
                nc.scalar.activation(
                    exp_t[:, 2 * m : 2 * m + 2, :],
                    spt[:, :, :],
                    mybir.ActivationFunctionType.Exp,
                    scale=SCALE,
                )

            # bg tiles: bg matmuls directly after m1 so their psum->sbuf copy
            # (scalar, queued between exp m1 and exp m2) never leaves ACT
            # idle, and exp m3 lands as early as possible -- the ring gate
            # scores-m1(t+1) <- exp-m3(t) is what sets the tile period.
            scores_m(0)
            scores_m(1)
            if bg is not None:
                bg_dst, bg_src = emit_bg_mms(bg)
                copy_to(nc.scalar, bg_dst, bg_src)
                scores_m(2)
                if t >= LAG:
                    emit_attn_half(t - LAG, 0)
                scores_m(3)
                if t >= LAG:
                    emit_attn_half(t - LAG, 1)
            else:
                if t >= LAG:
                    emit_attn_half(t - LAG, 0)
                scores_m(2)
                if t >= LAG:
                    emit_attn_half(t - LAG, 1)
                scores_m(3)

            # head-sum tree entirely on vector: GPSIMD shares an SBUF port
            # pair with DVE under an exclusive lock, so ANY gpsimd op stalls
            # concurrent 2-port DVE ops mid-instruction -- keep gpsimd idle.
            s0123 = sm.tile([NC_, CH, QI], bf16, tag="s01", name=f"s01_{t}")
            nc.vector.tensor_add(s0123, exp_t[:, 0:4, :], exp_t[:, 4:8, :])
            s2 = sm.tile([NC_, 2, QI], bf16, tag="s23", name=f"s23_{t}")
            nc.vector.tensor_add(s2, s0123[:, 0:2, :], s0123[:, 2:4, :])
            r = sm.tile([NC_, QI], bf16, tag="r", name=f"r{t}")
            if RSUM is not None:
                # fused r = 1/(a+b): saves the separate fp32 fold each tile
                nc.vector._custom_dve(
                    RSUM,
                    out=r,
                    in0=s2[:, 0, :],
                    in1=s2[:, 1, :],
                    s0=_RC["s0"],
                    s1=_RC["s1"],
                )
            else:
                ssum = sm.tile([NC_, QI], fp32, tag="ssum", name=f"ss{t}")
                nc.vector.tensor_add(ssum, s2[:, 0, :], s2[:, 1, :])
                nc.vector._custom_dve(
                    _RF,
                    out=r,
                    in0=ssum,
                    s0=_RC["s0"],
                    s1=_RC["s1"],
                    imm2=_RC["imm2"],
                )

            # normalize all 8 heads in one mul, r broadcast over the head dim
            # via a 0-stride AP (2x engages; clean now that gpsimd is idle).
            # Last tile: two halves so drain attn matmuls start after half A.
            pr = pp.tile([NC_, H, QI], bf16, tag="probs", bufs=4, name=f"pr{t}")
            if t == NKJ - 1:
                rb = bass.AP(
                    r.tensor, r.offset, [list(r.ap[0]), [0, 4], list(r.ap[-1])]
                )
                nc.vector.tensor_mul(pr[:, 0:4, :], exp_t[:, 0:4, :], rb)
                nc.vector.tensor_mul(pr[:, 4:8, :], exp_t[:, 4:8, :], rb)
            else:
                rb = bass.AP(
                    r.tensor, r.offset, [list(r.ap[0]), [0, H], list(r.ap[-1])]
                )
                nc.vector.tensor_mul(pr, exp_t, rb)
            prs_hist[t] = pr

        # ---- drain: attn tail, attnT copies, O projection ----
        emit_attn_half(NKJ - 2, 0)
        emit_attn_half(NKJ - 2, 1)
        ceng = [nc.vector, nc.scalar, nc.scalar, nc.vector]
        pr15 = prs_hist[NKJ - 1]
        for dc in range(4):
            for h in (2 * dc, 2 * dc + 1):
                po = (h % 2) * 64
                nc.tensor.matmul(
                    at[dc][po : po + 64, :QI],
                    lhsT=vs[:, NKJ - 1, h * 64 : (h + 1) * 64],
                    rhs=pr15[:, h, :],
                    start=False,
                    stop=True,
                )
            copy_to(ceng[dc], attnT[:, dc, :], at[dc][:, :QI])

        # O proj, c-outer so each chunk starts as soon as attnT[:, c] lands
        ots = [pa.tile([NC_, D], fp32, tag="attn", name=f"ot{m}") for m in range(4)]
        for c in range(CH):
            for m in range(4):
                if with_bias and c == 0:
                    bias_mm(ots[m], "bo", None)
                nc.tensor.matmul(
                    ots[m],
                    lhsT=attnT[:, c, m * 128 : (m + 1) * 128],
                    rhs=wsb["wot"][:, c, :],
                    start=(c == 0 and not with_bias),
                    stop=(c == CH - 1),
                )
        out_v = out_d.rearrange("(m p) o -> p m o", p=NC_)
        for m in range(4):
            eng = nc.scalar if m % 2 == 0 else nc.vector
            copy_to(eng, outsb[:, m, :], ots[m])
            # two half-DMAs land on separate queues -> shorter drain
            nc.sync.dma_start(
                out=out_v[:, m, 0:256], in_=outsb[:, m, 0:256]
            )
            nc.sync.dma_start(
                out=out_v[:, m, 256:512], in_=outsb[:, m, 256:512]
            )

    nc.compile()
    return nc


_CACHE = {}


def kernel(Q, K, V, w_q, b_q, w_k, b_k, w_v, b_v, w_o, b_o, _trace=False):
    import ml_dtypes
    from concourse import bass_utils

    bf = ml_dtypes.bfloat16
    Q = np.asarray(Q, np.float32)
    K = np.asarray(K, np.float32)
    V = np.asarray(V, np.float32)
    with_bias = any(
        np.any(np.asarray(b) != 0) for b in (b_q, b_k, b_v, b_o)
    )

    if ("nc", with_bias) not in _CACHE:
        _CACHE[("nc", with_bias)] = _build(with_bias)
    nc = _CACHE[("nc", with_bias)]

    wmaps = {
        "wqt": _chunk(np.asarray(w_q, np.float32).T, bf),
        "wkt": _chunk(np.asarray(w_k, np.float32).T, bf),
        "wvt": _chunk(np.asarray(w_v, np.float32).T, bf),
        "wot": _chunk(np.asarray(w_o, np.float32).T, bf),
    }
    if with_bias:
        for n, b in (("bq", b_q), ("bk", b_k), ("bv", b_v), ("bo", b_o)):
            wmaps[n] = np.ascontiguousarray(
                np.asarray(b, np.float32).reshape(1, D)
            ).astype(bf)

    in_maps = []
    for c in range(NCORES):
        b = c // CPB
        s0 = (c % CPB) * QI
        in_maps.append(
            dict(
                wmaps,
                qt=_chunk(Q[b, s0 : s0 + QI, :].T, bf),
                kt=_chunk(K[b].T, bf),
                vt=_chunk(V[b].T, bf),
            )
        )

    res = bass_utils.run_bass_kernel_spmd(
        nc, in_maps, core_ids=list(range(NCORES)), trace=_trace
    )

    out = np.empty((B, S, D), np.float32)
    for c in range(NCORES):
        b = c // CPB
        s0 = (c % CPB) * QI
        out[b, s0 : s0 + QI, :] = res.results[c]["out"]
    if _trace:
        kernel._last_results = res
    return out
